# Initial kernel scaffold
#
"""Self-contained Trainium2 Bass kernel for nn_Attention_62560493633940.

Sharding: 16 heads split across 8 cores (2 q-heads + their shared kv-head
per core, tensor parallel); x / pos replicated; per-core partial output
projections (over that core's 128 o-columns) summed on host.

Math note: pos_logits[h,q,k] = a[q,h] - a[k,h] + bh[h] with a = p @ Wh.T,
so softmax_k(pos_logits) is independent of q (shift invariance) ->
pos_attn is a rank-1 per-head key distribution; no [t,t,PF] diff tensor.
Both softmax row-sums are exactly 1, so the re-normalization in the
reference is an identity and the gate mix is (1-g)*attn + g*pos_attn.
"""
import sys

if '/opt/trn_rl_repo' not in sys.path:
    sys.path.insert(0, '/opt/trn_rl_repo')

import numpy as np

import concourse.bass as bass
import concourse.bacc as bacc
import concourse.tile as tile
import concourse.mybir as mybir
from concourse import bass_utils
from concourse.masks import make_identity

F32 = mybir.dt.float32
F16 = mybir.dt.float16

T = 1024      # sequence length
DIM = 1024    # model dim
H = 16        # heads
KVH = 4       # kv heads
HD = 64       # head dim
PD = 64       # pos dim
PF = 128      # pos feature dim
BASE = 10000.0
NC = 8        # cores

_CACHE = {}


def _build_program(reps=1):
    nc = bacc.Bacc("TRN2")

    # ---- DRAM parameters (per-core data arrives via in_maps) ----
    xT_d = nc.declare_dram_parameter("xT", [DIM, T], F16, isOutput=False)
    wq_d = nc.declare_dram_parameter("wq", [DIM, 128], F16, isOutput=False)
    wkv_d = nc.declare_dram_parameter("wkv", [DIM, 128], F16, isOutput=False)
    wo_d = nc.declare_dram_parameter("wo", [128, DIM], F16, isOutput=False)
    posT_d = nc.declare_dram_parameter("posT", [PD, T], F16, isOutput=False)
    wp1T_d = nc.declare_dram_parameter("wp1T", [PD, PD], F16, isOutput=False)
    wp2T_d = nc.declare_dram_parameter("wp2T", [PD, PF], F16, isOutput=False)
    whT2_d = nc.declare_dram_parameter("whT2", [PF, 32], F16, isOutput=False)
    bp1_d = nc.declare_dram_parameter("bp1c", [PD, 1], F32, isOutput=False)
    bp2_d = nc.declare_dram_parameter("bp2c", [PF, 1], F32, isOutput=False)
    g2a_d = nc.declare_dram_parameter("gate2a", [2, 1], F32, isOutput=False)
    g2b_d = nc.declare_dram_parameter("gate2b", [33, 1], F32, isOutput=False)
    tabc_d = nc.declare_dram_parameter("tabc", [32, T], F16, isOutput=False)
    tabs_d = nc.declare_dram_parameter("tabs", [64, T], F16, isOutput=False)
    outp_d = nc.declare_dram_parameter("outp", [T, DIM], F16, isOutput=True)

    ExpF = mybir.ActivationFunctionType.Exp
    ReluF = mybir.ActivationFunctionType.Relu
    IdentF = mybir.ActivationFunctionType.Identity
    SigF = mybir.ActivationFunctionType.Sigmoid
    CopyF = mybir.ActivationFunctionType.Copy

    with tile.TileContext(nc) as tc:
        with tc.tile_pool(name="cst", bufs=1) as cst, \
             tc.tile_pool(name="wk", bufs=1) as wk, \
             tc.tile_pool(name="eP", bufs=6) as eP, \
             tc.tile_pool(name="vP", bufs=8) as vP, \
             tc.tile_pool(name="psW", bufs=2, space="PSUM") as psW, \
             tc.tile_pool(name="psA", bufs=3, space="PSUM") as psA, \
             tc.tile_pool(name="psS", bufs=1, space="PSUM") as psS:

            for _rep in range(reps):
                # ---- constants into SBUF ----
                xT_sb = cst.tile([128, 8, T], F16)
                xT_ap = xT_d.ap().rearrange("(k p) t -> p k t", p=128)
                nc.gpsimd.dma_start(out=xT_sb[:, 0:1, 0:512],
                                    in_=xT_ap[:, 0:1, 0:512])
                nc.gpsimd.dma_start(out=xT_sb[:, 0:1, 512:1024],
                                    in_=xT_ap[:, 0:1, 512:1024])
                for kk in range(1, 8):
                    nc.gpsimd.dma_start(out=xT_sb[:, kk:kk + 1, :],
                                        in_=xT_ap[:, kk:kk + 1, :])
                wq_sb = cst.tile([128, 8, 128], F16)
                nc.sync.dma_start(out=wq_sb[:],
                                  in_=wq_d.ap().rearrange("(k p) m -> p k m", p=128))
                wkv_sb = cst.tile([128, 8, 128], F16)
                nc.sync.dma_start(out=wkv_sb[:],
                                  in_=wkv_d.ap().rearrange("(k p) m -> p k m", p=128))
                posT_sb = cst.tile([PD, T], F16)
                nc.gpsimd.dma_start(out=posT_sb[:], in_=posT_d.ap())
                wp1T_sb = cst.tile([PD, PD], F16)
                nc.gpsimd.dma_start(out=wp1T_sb[:], in_=wp1T_d.ap())
                wp2T_sb = cst.tile([PD, PF], F16)
                nc.gpsimd.dma_start(out=wp2T_sb[:], in_=wp2T_d.ap())
                whT2_sb = cst.tile([PF, 32], F16)
                nc.gpsimd.dma_start(out=whT2_sb[:], in_=whT2_d.ap())
                bp1_sb = cst.tile([PD, 1], F32)
                nc.sync.dma_start(out=bp1_sb[:], in_=bp1_d.ap())
                bp2_sb = cst.tile([PF, 1], F32)
                nc.sync.dma_start(out=bp2_sb[:], in_=bp2_d.ap())
                g2a_sb = cst.tile([2, 1], F32)
                nc.sync.dma_start(out=g2a_sb[:], in_=g2a_d.ap())
                g2b_sb = cst.tile([33, 1], F32)
                nc.sync.dma_start(out=g2b_sb[:], in_=g2b_d.ap())
                tabc_sb = cst.tile([128, T], F16)
                nc.sync.dma_start(out=tabc_sb[0:32, :], in_=tabc_d.ap())
                for bb in range(1, 4):
                    nc.sync.dma_start(out=tabc_sb[32 * bb:32 * bb + 32, :],
                                      in_=tabc_sb[0:32, :])
                tabs_sb = cst.tile([128, T], F16)
                nc.sync.dma_start(out=tabs_sb[0:64, :], in_=tabs_d.ap())
                nc.sync.dma_start(out=tabs_sb[64:128, :], in_=tabs_sb[0:64, :])

                ones_f = cst.tile([1, 128], F32)
                nc.vector.memset(ones_f, 1.0)
                ones_r = cst.tile([1, 128], F16)
                nc.vector.tensor_scalar_mul(ones_r[:], ones_f[:], 1.0)
                onescol_f = cst.tile([128, 2], F32)
                nc.vector.memset(onescol_f, 1.0)
                onescol_r = cst.tile([128, 2], F16)
                nc.vector.tensor_scalar_mul(onescol_r[:], onescol_f[:], 1.0)
                id64f = cst.tile([64, 64], F32)
                make_identity(nc, id64f)
                id64r = cst.tile([64, 64], F16)
                nc.vector.tensor_scalar_mul(id64r[:], id64f[:], 1.0)

                # ---- q / kv projections (T layout: out rows = proj dims) ----
                qraw = psW.tile([128, T], F32, tag="wide")
                kvraw = psW.tile([128, T], F32, tag="wide")
                for n in range(2):
                    for k in range(8):
                        nc.tensor.matmul(qraw[:, 512 * n:512 * n + 512],
                                         wq_sb[:, k, :],
                                         xT_sb[:, k, 512 * n:512 * n + 512],
                                         start=(k == 0), stop=(k == 7))
                    for k in range(8):
                        nc.tensor.matmul(kvraw[:, 512 * n:512 * n + 512],
                                         wkv_sb[:, k, :],
                                         xT_sb[:, k, 512 * n:512 * n + 512],
                                         start=(k == 0), stop=(k == 7))

                # ---- RoPE on q (rows: [x1_h0, x2_h0, x1_h1, x2_h1] in 32-blocks)
                T1 = wk.tile([128, T], F16)
                T2 = wk.tile([128, T], F16)
                T2s = wk.tile([128, T], F16)
                qT = wk.tile([128, T], F16)
                for n in range(2):
                    c0 = 512 * n
                    nc.vector.tensor_mul(T1[:, c0:c0 + 512], qraw[:, c0:c0 + 512],
                                         tabc_sb[:, c0:c0 + 512])
                    nc.vector.tensor_mul(T2[:, c0:c0 + 512], qraw[:, c0:c0 + 512],
                                         tabs_sb[:, c0:c0 + 512])
                    for b in range(4):
                        sr = (b // 2) * 64 + (1 - (b % 2)) * 32
                        ds = (b // 2) * 64 + (b % 2) * 32
                        nc.sync.dma_start(out=T2s[ds:ds + 32, c0:c0 + 512],
                                          in_=T2[sr:sr + 32, c0:c0 + 512])
                    nc.vector.tensor_add(qT[:, c0:c0 + 512], T1[:, c0:c0 + 512],
                                         T2s[:, c0:c0 + 512])

                # ---- RoPE on k (kvraw rows 0:64) + duplicate into rows 64:128
                T1k = wk.tile([64, T], F16)
                T2k = wk.tile([64, T], F16)
                T2ks = wk.tile([64, T], F16)
                kT2 = wk.tile([128, T], F16)
                for n in range(2):
                    c0 = 512 * n
                    nc.vector.tensor_mul(T1k[:, c0:c0 + 512],
                                         kvraw[0:64, c0:c0 + 512],
                                         tabc_sb[0:64, c0:c0 + 512])
                    nc.vector.tensor_mul(T2k[:, c0:c0 + 512],
                                         kvraw[0:64, c0:c0 + 512],
                                         tabs_sb[0:64, c0:c0 + 512])
                    nc.sync.dma_start(out=T2ks[0:32, c0:c0 + 512],
                                      in_=T2k[32:64, c0:c0 + 512])
                    nc.sync.dma_start(out=T2ks[32:64, c0:c0 + 512],
                                      in_=T2k[0:32, c0:c0 + 512])
                    nc.vector.tensor_add(kT2[0:64, c0:c0 + 512],
                                         T1k[:, c0:c0 + 512],
                                         T2ks[:, c0:c0 + 512])
                    nc.sync.dma_start(out=kT2[64:128, c0:c0 + 512],
                                      in_=kT2[0:64, c0:c0 + 512])

                wo_sb = cst.tile([128, DIM], F16)
                nc.sync.dma_start(out=wo_sb[:], in_=wo_d.ap())

                # ---- v: copy vT out of PSUM, PE-transpose into v_aug (+ones col)
                vT_sb = wk.tile([64, T], F16)
                nc.vector.tensor_copy(vT_sb[:, 0:512], kvraw[64:128, 0:512])
                nc.vector.tensor_copy(vT_sb[:, 512:1024], kvraw[64:128, 512:1024])
                v_aug = []
                for m in range(8):
                    vtp = psS.tile([128, 64], F16, tag="sm")
                    nc.tensor.transpose(vtp[:], vT_sb[:, 128 * m:128 * m + 128],
                                        id64r[:])
                    va = vP.tile([128, 66], F16, tag="vaug")
                    nc.vector.tensor_copy(va[:, 0:64], vtp[:])
                    nc.vector.tensor_copy(va[:, 64:66], onescol_r[:])
                    v_aug.append(va)

                # ---- pos path ----
                pTr = wk.tile([PD, T], F16)
                for n in range(2):
                    pp = psS.tile([PD, 512], F32, tag="sm")
                    nc.tensor.matmul(pp[:], wp1T_sb[:],
                                     posT_sb[:, 512 * n:512 * n + 512],
                                     start=True, stop=True)
                    nc.scalar.activation(pTr[:, 512 * n:512 * n + 512], pp[:],
                                         ReluF, bias=bp1_sb[:, 0:1], scale=1.0)
                p2Tb = wk.tile([PF, T], F16)
                for n in range(2):
                    p2p = psS.tile([PF, 512], F32, tag="sm")
                    nc.tensor.matmul(p2p[:], wp2T_sb[:],
                                     pTr[:, 512 * n:512 * n + 512],
                                     start=True, stop=True)
                    nc.scalar.activation(p2Tb[:, 512 * n:512 * n + 512], p2p[:],
                                         IdentF, bias=bp2_sb[:, 0:1], scale=1.0)
                eposAll = wk.tile([128, 256], F16)
                aALL = psS.tile([128, 256], F32, tag="sm")
                for j in range(8):
                    nc.tensor.matmul(aALL[:, 32 * j:32 * j + 32],
                                     p2Tb[:, 128 * j:128 * j + 128],
                                     whT2_sb[:], start=True, stop=True)
                nc.scalar.activation(eposAll[:], aALL[:], ExpF, scale=-1.0)
                posout = psA.tile([32, 66], F32, tag="av")
                for j in range(8):
                    nc.tensor.matmul(posout[:], eposAll[:, 32 * j:32 * j + 32], v_aug[j][:],
                                     start=(j == 0), stop=(j == 7))
                recipZp = wk.tile([2, 1], F32)
                nc.vector.reciprocal(recipZp[:], posout[0:2, 64:65])
                e2a = wk.tile([2, 1], F32)
                nc.scalar.activation(e2a[:], g2a_sb[:], ExpF, scale=-1.0)
                e2a1 = wk.tile([2, 1], F32)
                nc.vector.tensor_scalar_add(e2a1[:], e2a[:], 1.0)
                sg2a = wk.tile([2, 1], F32)
                nc.vector.reciprocal(sg2a[:], e2a1[:])
                gz2 = wk.tile([2, 1], F32)
                nc.vector.tensor_mul(gz2[:], recipZp[:], sg2a[:])
                gpos2 = wk.tile([2, 64], F16)
                nc.vector.tensor_scalar_mul(gpos2[:], posout[0:2, 0:64], gz2[:, 0:1])
                gposTp = psA.tile([64, 2], F16, tag="av")
                nc.tensor.transpose(gposTp[:], gpos2[:], id64r[0:2, 0:2])
                gposT2 = wk.tile([128, 1], F32)
                nc.vector.tensor_copy(gposT2[0:64, :], gposTp[:, 0:1])
                nc.vector.tensor_copy(gposT2[64:128, :], gposTp[:, 1:2])
                e33 = wk.tile([33, 1], F32)
                nc.scalar.activation(e33[:], g2b_sb[:], ExpF, scale=-1.0)
                e331 = wk.tile([33, 1], F32)
                nc.vector.tensor_scalar_add(e331[:], e33[:], 1.0)
                r331 = wk.tile([33, 1], F32)
                nc.vector.reciprocal(r331[:], e331[:])
                sginv33 = wk.tile([33, 1], F32)
                nc.vector.tensor_mul(sginv33[:], e33[:], r331[:])

                # ---- attention per head ----
                oT = wk.tile([128, T], F16)
                zbS = wk.tile([128, T], F16)
                for i in range(2):
                    r = 64 * i
                    avh = [psA.tile([66, 512], F32, tag="av",
                                    name=f"avh{i}_0"),
                           psA.tile([66, 512], F32, tag="av",
                                    name=f"avh{i}_1")]
                    for m in range(8):
                        E = eP.tile([128, T], F16, tag="E")
                        S = psW.tile([128, T], F32, tag="wide")
                        for n in range(2):
                            nc.tensor.matmul(
                                S[:, 512 * n:512 * n + 512],
                                kT2[r:r + 64, 128 * m:128 * m + 128],
                                qT[r:r + 64, 512 * n:512 * n + 512],
                                start=True, stop=True)
                        nc.scalar.activation(E[:], S[:], ExpF, scale=0.125)
                        for n in range(2):
                            nc.tensor.matmul(avh[n][:],
                                             v_aug[m][:],
                                             E[:, 512 * n:512 * n + 512],
                                             start=(m == 0), stop=(m == 7))
                    for n in range(2):
                        c0 = 512 * n
                        recipZ = wk.tile([1, 512], F32, tag=f"rz{i}{n}")
                        nc.vector.reciprocal(recipZ[:], avh[n][64:65, :])
                        recipZg = wk.tile([1, 512], F16, tag=f"rg{i}{n}")
                        nc.vector.tensor_scalar_mul(
                            recipZg[:], recipZ[:],
                            sginv33[32 * i:32 * i + 1, 0:1])
                        zb = psS.tile([64, 512], F32, tag="sm")
                        nc.tensor.matmul(zb[:], ones_r[:, 0:64],
                                         recipZg[:],
                                         start=True, stop=True)
                        nc.vector.tensor_copy(zbS[r:r + 64, c0:c0 + 512], zb[:])
                        nc.vector.tensor_mul(oT[r:r + 64, c0:c0 + 512],
                                             avh[n][0:64, :],
                                             zbS[r:r + 64, c0:c0 + 512])
                        nc.vector.tensor_scalar_add(
                            oT[r:r + 64, c0:c0 + 512],
                            oT[r:r + 64, c0:c0 + 512],
                            gposT2[r:r + 64, 0:1])

                # ---- output projection (partial over this core's 128 o-cols)
                outp_ap = outp_d.ap()
                for j in range(8):
                    outS = eP.tile([128, DIM], F16, tag="outS")
                    po = psW.tile([128, DIM], F32, tag="wide")
                    for n in range(2):
                        nc.tensor.matmul(po[:, 512 * n:512 * n + 512],
                                         oT[:, 128 * j:128 * j + 128],
                                         wo_sb[:, 512 * n:512 * n + 512],
                                         start=True, stop=True)
                    if j % 2 == 0:
                        nc.scalar.copy(outS[:], po[:])
                    else:
                        nc.vector.tensor_copy(outS[:], po[:])
                    nc.sync.dma_start(
                        out=outp_ap[128 * j:128 * j + 128, :], in_=outS[:])

    nc.compile()
    return nc


def _host_inputs(inputs):
    """Per-core in_maps from the full inputs."""
    x = np.asarray(inputs["x"], np.float32)
    pos = np.asarray(inputs["pos"], np.float32)
    Wq = np.asarray(inputs["Wq"], np.float32)
    Wk = np.asarray(inputs["Wk"], np.float32)
    Wv = np.asarray(inputs["Wv"], np.float32)
    Wo = np.asarray(inputs["Wo"], np.float32)
    bo = np.asarray(inputs["bo"], np.float32)
    Wp1 = np.asarray(inputs["Wp1"], np.float32)
    bp1 = np.asarray(inputs["bp1"], np.float32)
    Wp2 = np.asarray(inputs["Wp2"], np.float32)
    bp2 = np.asarray(inputs["bp2"], np.float32)
    Wh = np.asarray(inputs["Wh"], np.float32)
    gate = np.asarray(inputs["gate"], np.float32)

    xT = np.ascontiguousarray(x[0].T).astype(np.float16)
    posT = np.ascontiguousarray(pos[0].T).astype(np.float16)
    wp1T = np.ascontiguousarray(Wp1.T).astype(np.float16)
    wp2T = np.ascontiguousarray(Wp2.T).astype(np.float16)
    bp1c = bp1.reshape(PD, 1).copy()
    bp2c = bp2.reshape(PF, 1).copy()

    # RoPE tables in transposed layout, tiled 4x along partitions
    j = np.arange(HD // 2, dtype=np.float32)
    theta = (BASE ** (-2.0 * j / HD)).astype(np.float32)
    freqs = np.arange(T, dtype=np.float32)[:, None] * theta  # [T, 32]
    cosT = np.ascontiguousarray(np.cos(freqs).T.astype(np.float32))
    sinT = np.ascontiguousarray(np.sin(freqs).T.astype(np.float32))
    tabc = cosT.astype(np.float16)
    tabs = np.concatenate([sinT, -sinT], 0).astype(np.float16)

    in_maps = []
    for c in range(NC):
        g = c // 2
        wq_c = np.ascontiguousarray(Wq[128 * c:128 * c + 128, :].T).astype(np.float16)
        wkv_c = np.ascontiguousarray(
            np.concatenate([Wk[64 * g:64 * g + 64, :],
                            Wv[64 * g:64 * g + 64, :]], 0).T).astype(np.float16)
        wo_c = np.ascontiguousarray(Wo[:, 128 * c:128 * c + 128].T).astype(np.float16)
        whT2_c = np.zeros((PF, 32), np.float16)
        whT2_c[:, 0:2] = Wh[2 * c:2 * c + 2, :].T
        g2a = gate[2 * c:2 * c + 2].reshape(2, 1).copy()
        g2b = np.zeros((33, 1), np.float32)
        g2b[0, 0] = gate[2 * c]
        g2b[32, 0] = gate[2 * c + 1]
        in_maps.append({
            "xT": xT, "wq": wq_c, "wkv": wkv_c, "wo": wo_c,
            "posT": posT, "wp1T": wp1T, "wp2T": wp2T,
            "whT2": whT2_c, "bp1c": bp1c, "bp2c": bp2c,
            "gate2a": g2a, "gate2b": g2b, "tabc": tabc, "tabs": tabs,
        })
    return in_maps


def get_program(reps=1):
    key = f"nc{reps}"
    if key not in _CACHE:
        _CACHE[key] = _build_program(reps)
    return _CACHE[key]


def kernel(**inputs) -> np.ndarray:
    nc = get_program()
    in_maps = _host_inputs(inputs)
    res = bass_utils.run_bass_kernel_spmd(nc, in_maps, list(range(NC)))
    out = np.zeros((T, DIM), np.float32)
    for c in range(NC):
        out += res.results[c]["outp"].astype(np.float32)
    out += np.asarray(inputs["bo"], np.float32)
    return out.reshape(1, T, DIM)



# revision 1
# speedup vs baseline: 3.1081x; 3.1081x over previous
"""Self-contained Trainium2 Bass kernel for nn_Attention_62560493633940.

Sharding: 16 heads split across 8 cores (2 q-heads + their shared kv-head
per core, tensor parallel); x / pos replicated; per-core partial output
projections (over that core's 128 o-columns) summed on host.

Math note: pos_logits[h,q,k] = a[q,h] - a[k,h] + bh[h] with a = p @ Wh.T,
so softmax_k(pos_logits) is independent of q (shift invariance) ->
pos_attn is a rank-1 per-head key distribution; no [t,t,PF] diff tensor.
Both softmax row-sums are exactly 1, so the re-normalization in the
reference is an identity and the gate mix is (1-g)*attn + g*pos_attn.
"""
import sys

if '/opt/trn_rl_repo' not in sys.path:
    sys.path.insert(0, '/opt/trn_rl_repo')

import numpy as np

import concourse.bass as bass
import concourse.bacc as bacc
import concourse.tile as tile
import concourse.mybir as mybir
from concourse import bass_utils
from concourse.masks import make_identity

F32 = mybir.dt.float32
F16 = mybir.dt.float16

T = 1024      # sequence length
DIM = 1024    # model dim
H = 16        # heads
KVH = 4       # kv heads
HD = 64       # head dim
PD = 64       # pos dim
PF = 128      # pos feature dim
BASE = 10000.0
NC = 8        # cores

_CACHE = {}


def _build_program(reps=1):
    nc = bacc.Bacc("TRN2")

    # ---- DRAM parameters (per-core data arrives via in_maps) ----
    xT_d = nc.declare_dram_parameter("xT", [DIM, T], F16, isOutput=False)
    wq_d = nc.declare_dram_parameter("wq", [DIM, 128], F16, isOutput=False)
    wkv_d = nc.declare_dram_parameter("wkv", [DIM, 128], F16, isOutput=False)
    wo_d = nc.declare_dram_parameter("wo", [128, DIM], F16, isOutput=False)
    posT_d = nc.declare_dram_parameter("posT", [PD, T], F16, isOutput=False)
    wp1T_d = nc.declare_dram_parameter("wp1T", [PD, PD], F16, isOutput=False)
    wp2T_d = nc.declare_dram_parameter("wp2T", [PD, PF], F16, isOutput=False)
    whT2_d = nc.declare_dram_parameter("whT2", [PF, 32], F16, isOutput=False)
    bp1_d = nc.declare_dram_parameter("bp1c", [PD, 1], F32, isOutput=False)
    bp2_d = nc.declare_dram_parameter("bp2c", [PF, 1], F32, isOutput=False)
    g2a_d = nc.declare_dram_parameter("gate2a", [2, 1], F32, isOutput=False)
    g2b_d = nc.declare_dram_parameter("gate2b", [33, 1], F32, isOutput=False)
    tabc_d = nc.declare_dram_parameter("tabc", [32, T], F16, isOutput=False)
    tabs_d = nc.declare_dram_parameter("tabs", [64, T], F16, isOutput=False)
    outp_d = nc.declare_dram_parameter("outp", [T, DIM], F16, isOutput=True)

    ExpF = mybir.ActivationFunctionType.Exp
    ReluF = mybir.ActivationFunctionType.Relu
    IdentF = mybir.ActivationFunctionType.Identity
    SigF = mybir.ActivationFunctionType.Sigmoid
    CopyF = mybir.ActivationFunctionType.Copy

    with tile.TileContext(nc) as tc:
        with tc.tile_pool(name="cst", bufs=1) as cst, \
             tc.tile_pool(name="wk", bufs=1) as wk, \
             tc.tile_pool(name="eP", bufs=6) as eP, \
             tc.tile_pool(name="vP", bufs=8) as vP, \
             tc.tile_pool(name="psW", bufs=2, space="PSUM") as psW, \
             tc.tile_pool(name="psA", bufs=3, space="PSUM") as psA, \
             tc.tile_pool(name="psS", bufs=1, space="PSUM") as psS:

            for _rep in range(reps):
                # ---- constants into SBUF ----
                xT_sb = cst.tile([128, 8, T], F16)
                xT_ap = xT_d.ap().rearrange("(k p) t -> p k t", p=128)
                nc.gpsimd.dma_start(out=xT_sb[:, 0:1, 0:512],
                                    in_=xT_ap[:, 0:1, 0:512])
                nc.gpsimd.dma_start(out=xT_sb[:, 0:1, 512:1024],
                                    in_=xT_ap[:, 0:1, 512:1024])
                for kk in range(1, 8):
                    nc.gpsimd.dma_start(out=xT_sb[:, kk:kk + 1, :],
                                        in_=xT_ap[:, kk:kk + 1, :])
                wq_sb = cst.tile([128, 8, 128], F16)
                nc.sync.dma_start(out=wq_sb[:],
                                  in_=wq_d.ap().rearrange("(k p) m -> p k m", p=128))
                wkv_sb = cst.tile([128, 8, 128], F16)
                nc.sync.dma_start(out=wkv_sb[:],
                                  in_=wkv_d.ap().rearrange("(k p) m -> p k m", p=128))
                posT_sb = cst.tile([PD, T], F16)
                nc.gpsimd.dma_start(out=posT_sb[:], in_=posT_d.ap())
                wp1T_sb = cst.tile([PD, PD], F16)
                nc.gpsimd.dma_start(out=wp1T_sb[:], in_=wp1T_d.ap())
                wp2T_sb = cst.tile([PD, PF], F16)
                nc.gpsimd.dma_start(out=wp2T_sb[:], in_=wp2T_d.ap())
                whT2_sb = cst.tile([PF, 32], F16)
                nc.gpsimd.dma_start(out=whT2_sb[:], in_=whT2_d.ap())
                bp1_sb = cst.tile([PD, 1], F32)
                nc.sync.dma_start(out=bp1_sb[:], in_=bp1_d.ap())
                bp2_sb = cst.tile([PF, 1], F32)
                nc.sync.dma_start(out=bp2_sb[:], in_=bp2_d.ap())
                g2a_sb = cst.tile([2, 1], F32)
                nc.sync.dma_start(out=g2a_sb[:], in_=g2a_d.ap())
                g2b_sb = cst.tile([33, 1], F32)
                nc.sync.dma_start(out=g2b_sb[:], in_=g2b_d.ap())
                tabc_sb = cst.tile([128, T], F16)
                nc.sync.dma_start(out=tabc_sb[0:32, :], in_=tabc_d.ap())
                for bb in range(1, 4):
                    nc.sync.dma_start(out=tabc_sb[32 * bb:32 * bb + 32, :],
                                      in_=tabc_sb[0:32, :])
                tabs_sb = cst.tile([128, T], F16)
                nc.sync.dma_start(out=tabs_sb[0:64, :], in_=tabs_d.ap())
                nc.sync.dma_start(out=tabs_sb[64:128, :], in_=tabs_sb[0:64, :])

                ones_f = cst.tile([1, 128], F32)
                nc.vector.memset(ones_f, 1.0)
                ones_r = cst.tile([1, 128], F16)
                nc.vector.tensor_scalar_mul(ones_r[:], ones_f[:], 1.0)
                onescol_f = cst.tile([128, 2], F32)
                nc.vector.memset(onescol_f, 1.0)
                onescol_r = cst.tile([128, 2], F16)
                nc.vector.tensor_scalar_mul(onescol_r[:], onescol_f[:], 1.0)
                id64f = cst.tile([64, 64], F32)
                make_identity(nc, id64f)
                id64r = cst.tile([64, 64], F16)
                nc.vector.tensor_scalar_mul(id64r[:], id64f[:], 1.0)

                # ---- q / kv projections (T layout: out rows = proj dims) ----
                qraw = psW.tile([128, T], F32, tag="wide")
                kvraw = psW.tile([128, T], F32, tag="wide")
                for n in range(2):
                    for k in range(8):
                        nc.tensor.matmul(qraw[:, 512 * n:512 * n + 512],
                                         wq_sb[:, k, :],
                                         xT_sb[:, k, 512 * n:512 * n + 512],
                                         start=(k == 0), stop=(k == 7))
                    for k in range(8):
                        nc.tensor.matmul(kvraw[:, 512 * n:512 * n + 512],
                                         wkv_sb[:, k, :],
                                         xT_sb[:, k, 512 * n:512 * n + 512],
                                         start=(k == 0), stop=(k == 7))

                # ---- RoPE on q (rows: [x1_h0, x2_h0, x1_h1, x2_h1] in 32-blocks)
                T1 = wk.tile([128, T], F16)
                T2 = wk.tile([128, T], F16)
                T2s = wk.tile([128, T], F16)
                qT = wk.tile([128, T], F16)
                for n in range(2):
                    c0 = 512 * n
                    nc.vector.tensor_mul(T1[:, c0:c0 + 512], qraw[:, c0:c0 + 512],
                                         tabc_sb[:, c0:c0 + 512])
                    nc.vector.tensor_mul(T2[:, c0:c0 + 512], qraw[:, c0:c0 + 512],
                                         tabs_sb[:, c0:c0 + 512])
                    for b in range(4):
                        sr = (b // 2) * 64 + (1 - (b % 2)) * 32
                        ds = (b // 2) * 64 + (b % 2) * 32
                        nc.sync.dma_start(out=T2s[ds:ds + 32, c0:c0 + 512],
                                          in_=T2[sr:sr + 32, c0:c0 + 512])
                    nc.vector.tensor_add(qT[:, c0:c0 + 512], T1[:, c0:c0 + 512],
                                         T2s[:, c0:c0 + 512])

                # ---- RoPE on k (kvraw rows 0:64) + duplicate into rows 64:128
                T1k = wk.tile([64, T], F16)
                T2k = wk.tile([64, T], F16)
                T2ks = wk.tile([64, T], F16)
                kT2 = wk.tile([128, T], F16)
                for n in range(2):
                    c0 = 512 * n
                    nc.vector.tensor_mul(T1k[:, c0:c0 + 512],
                                         kvraw[0:64, c0:c0 + 512],
                                         tabc_sb[0:64, c0:c0 + 512])
                    nc.vector.tensor_mul(T2k[:, c0:c0 + 512],
                                         kvraw[0:64, c0:c0 + 512],
                                         tabs_sb[0:64, c0:c0 + 512])
                    nc.sync.dma_start(out=T2ks[0:32, c0:c0 + 512],
                                      in_=T2k[32:64, c0:c0 + 512])
                    nc.sync.dma_start(out=T2ks[32:64, c0:c0 + 512],
                                      in_=T2k[0:32, c0:c0 + 512])
                    nc.vector.tensor_add(kT2[0:64, c0:c0 + 512],
                                         T1k[:, c0:c0 + 512],
                                         T2ks[:, c0:c0 + 512])
                    nc.sync.dma_start(out=kT2[64:128, c0:c0 + 512],
                                      in_=kT2[0:64, c0:c0 + 512])

                wo_sb = cst.tile([128, DIM], F16)
                nc.sync.dma_start(out=wo_sb[:], in_=wo_d.ap())

                # ---- v: copy vT out of PSUM, PE-transpose into v_aug (+ones col)
                vT_sb = wk.tile([64, T], F16)
                nc.vector.tensor_copy(vT_sb[:, 0:512], kvraw[64:128, 0:512])
                nc.vector.tensor_copy(vT_sb[:, 512:1024], kvraw[64:128, 512:1024])
                v_aug = []
                for m in range(8):
                    vtp = psS.tile([128, 64], F16, tag="sm")
                    nc.tensor.transpose(vtp[:], vT_sb[:, 128 * m:128 * m + 128],
                                        id64r[:])
                    va = vP.tile([128, 66], F16, tag="vaug")
                    nc.vector.tensor_copy(va[:, 0:64], vtp[:])
                    nc.vector.tensor_copy(va[:, 64:66], onescol_r[:])
                    v_aug.append(va)

                # ---- pos path ----
                pTr = wk.tile([PD, T], F16)
                for n in range(2):
                    pp = psS.tile([PD, 512], F32, tag="sm")
                    nc.tensor.matmul(pp[:], wp1T_sb[:],
                                     posT_sb[:, 512 * n:512 * n + 512],
                                     start=True, stop=True)
                    nc.scalar.activation(pTr[:, 512 * n:512 * n + 512], pp[:],
                                         ReluF, bias=bp1_sb[:, 0:1], scale=1.0)
                p2Tb = wk.tile([PF, T], F16)
                for n in range(2):
                    p2p = psS.tile([PF, 512], F32, tag="sm")
                    nc.tensor.matmul(p2p[:], wp2T_sb[:],
                                     pTr[:, 512 * n:512 * n + 512],
                                     start=True, stop=True)
                    nc.scalar.activation(p2Tb[:, 512 * n:512 * n + 512], p2p[:],
                                         IdentF, bias=bp2_sb[:, 0:1], scale=1.0)
                eposAll = wk.tile([128, 256], F16)
                aALL = psS.tile([128, 256], F32, tag="sm")
                for j in range(8):
                    nc.tensor.matmul(aALL[:, 32 * j:32 * j + 32],
                                     p2Tb[:, 128 * j:128 * j + 128],
                                     whT2_sb[:], start=True, stop=True)
                nc.scalar.activation(eposAll[:], aALL[:], ExpF, scale=-1.0)
                posout = psA.tile([32, 66], F32, tag="av")
                for j in range(8):
                    nc.tensor.matmul(posout[:], eposAll[:, 32 * j:32 * j + 32], v_aug[j][:],
                                     start=(j == 0), stop=(j == 7))
                recipZp = wk.tile([2, 1], F32)
                nc.vector.reciprocal(recipZp[:], posout[0:2, 64:65])
                e2a = wk.tile([2, 1], F32)
                nc.scalar.activation(e2a[:], g2a_sb[:], ExpF, scale=-1.0)
                e2a1 = wk.tile([2, 1], F32)
                nc.vector.tensor_scalar_add(e2a1[:], e2a[:], 1.0)
                sg2a = wk.tile([2, 1], F32)
                nc.vector.reciprocal(sg2a[:], e2a1[:])
                gz2 = wk.tile([2, 1], F32)
                nc.vector.tensor_mul(gz2[:], recipZp[:], sg2a[:])
                gpos2 = wk.tile([2, 64], F16)
                nc.vector.tensor_scalar_mul(gpos2[:], posout[0:2, 0:64], gz2[:, 0:1])
                gposTp = psA.tile([64, 2], F16, tag="av")
                nc.tensor.transpose(gposTp[:], gpos2[:], id64r[0:2, 0:2])
                gposT2 = wk.tile([128, 1], F32)
                nc.vector.tensor_copy(gposT2[0:64, :], gposTp[:, 0:1])
                nc.vector.tensor_copy(gposT2[64:128, :], gposTp[:, 1:2])
                e33 = wk.tile([33, 1], F32)
                nc.scalar.activation(e33[:], g2b_sb[:], ExpF, scale=-1.0)
                e331 = wk.tile([33, 1], F32)
                nc.vector.tensor_scalar_add(e331[:], e33[:], 1.0)
                r331 = wk.tile([33, 1], F32)
                nc.vector.reciprocal(r331[:], e331[:])
                sginv33 = wk.tile([33, 1], F32)
                nc.vector.tensor_mul(sginv33[:], e33[:], r331[:])

                # ---- attention per head ----
                oT = wk.tile([128, T], F16)
                zbS = wk.tile([128, T], F16)
                for i in range(2):
                    r = 64 * i
                    avh = [psA.tile([66, 512], F32, tag="av",
                                    name=f"avh{i}_0"),
                           psA.tile([66, 512], F32, tag="av",
                                    name=f"avh{i}_1")]
                    for m in range(8):
                        E = eP.tile([128, T], F16, tag="E")
                        S = psW.tile([128, T], F32, tag="wide")
                        for n in range(2):
                            nc.tensor.matmul(
                                S[:, 512 * n:512 * n + 512],
                                kT2[r:r + 64, 128 * m:128 * m + 128],
                                qT[r:r + 64, 512 * n:512 * n + 512],
                                start=True, stop=True)
                        nc.scalar.activation(E[:], S[:], ExpF, scale=0.125)
                        for n in range(2):
                            nc.tensor.matmul(avh[n][:],
                                             v_aug[m][:],
                                             E[:, 512 * n:512 * n + 512],
                                             start=(m == 0), stop=(m == 7))
                    for n in range(2):
                        c0 = 512 * n
                        recipZ = wk.tile([1, 512], F32, tag=f"rz{i}{n}")
                        nc.vector.reciprocal(recipZ[:], avh[n][64:65, :])
                        recipZg = wk.tile([1, 512], F16, tag=f"rg{i}{n}")
                        nc.vector.tensor_scalar_mul(
                            recipZg[:], recipZ[:],
                            sginv33[32 * i:32 * i + 1, 0:1])
                        zb = psS.tile([64, 512], F32, tag="sm")
                        nc.tensor.matmul(zb[:], ones_r[:, 0:64],
                                         recipZg[:],
                                         start=True, stop=True)
                        nc.vector.tensor_copy(zbS[r:r + 64, c0:c0 + 512], zb[:])
                        nc.vector.tensor_mul(oT[r:r + 64, c0:c0 + 512],
                                             avh[n][0:64, :],
                                             zbS[r:r + 64, c0:c0 + 512])
                        nc.vector.tensor_scalar_add(
                            oT[r:r + 64, c0:c0 + 512],
                            oT[r:r + 64, c0:c0 + 512],
                            gposT2[r:r + 64, 0:1])

                # ---- output projection (partial over this core's 128 o-cols)
                outp_ap = outp_d.ap()
                for j in range(8):
                    outS = eP.tile([128, DIM], F16, tag="outS")
                    po = psW.tile([128, DIM], F32, tag="wide")
                    for n in range(2):
                        nc.tensor.matmul(po[:, 512 * n:512 * n + 512],
                                         oT[:, 128 * j:128 * j + 128],
                                         wo_sb[:, 512 * n:512 * n + 512],
                                         start=True, stop=True)
                    if j % 2 == 0:
                        nc.scalar.copy(outS[:], po[:])
                    else:
                        nc.vector.tensor_copy(outS[:], po[:])
                    nc.sync.dma_start(
                        out=outp_ap[128 * j:128 * j + 128, :], in_=outS[:])

    nc.compile()
    return nc


def _host_inputs(inputs):
    """Per-core in_maps from the full inputs."""
    x = np.asarray(inputs["x"], np.float32)
    pos = np.asarray(inputs["pos"], np.float32)
    Wq = np.asarray(inputs["Wq"], np.float32)
    Wk = np.asarray(inputs["Wk"], np.float32)
    Wv = np.asarray(inputs["Wv"], np.float32)
    Wo = np.asarray(inputs["Wo"], np.float32)
    bo = np.asarray(inputs["bo"], np.float32)
    Wp1 = np.asarray(inputs["Wp1"], np.float32)
    bp1 = np.asarray(inputs["bp1"], np.float32)
    Wp2 = np.asarray(inputs["Wp2"], np.float32)
    bp2 = np.asarray(inputs["bp2"], np.float32)
    Wh = np.asarray(inputs["Wh"], np.float32)
    gate = np.asarray(inputs["gate"], np.float32)

    xT = np.ascontiguousarray(x[0].T).astype(np.float16)
    posT = np.ascontiguousarray(pos[0].T).astype(np.float16)
    wp1T = np.ascontiguousarray(Wp1.T).astype(np.float16)
    wp2T = np.ascontiguousarray(Wp2.T).astype(np.float16)
    bp1c = bp1.reshape(PD, 1).copy()
    bp2c = bp2.reshape(PF, 1).copy()

    # RoPE tables in transposed layout, tiled 4x along partitions
    j = np.arange(HD // 2, dtype=np.float32)
    theta = (BASE ** (-2.0 * j / HD)).astype(np.float32)
    freqs = np.arange(T, dtype=np.float32)[:, None] * theta  # [T, 32]
    cosT = np.ascontiguousarray(np.cos(freqs).T.astype(np.float32))
    sinT = np.ascontiguousarray(np.sin(freqs).T.astype(np.float32))
    tabc = cosT.astype(np.float16)
    tabs = np.concatenate([sinT, -sinT], 0).astype(np.float16)

    in_maps = []
    for c in range(NC):
        g = c // 2
        wq_c = np.ascontiguousarray(Wq[128 * c:128 * c + 128, :].T).astype(np.float16)
        wkv_c = np.ascontiguousarray(
            np.concatenate([Wk[64 * g:64 * g + 64, :],
                            Wv[64 * g:64 * g + 64, :]], 0).T).astype(np.float16)
        wo_c = np.ascontiguousarray(Wo[:, 128 * c:128 * c + 128].T).astype(np.float16)
        whT2_c = np.zeros((PF, 32), np.float16)
        whT2_c[:, 0:2] = Wh[2 * c:2 * c + 2, :].T
        g2a = gate[2 * c:2 * c + 2].reshape(2, 1).copy()
        g2b = np.zeros((33, 1), np.float32)
        g2b[0, 0] = gate[2 * c]
        g2b[32, 0] = gate[2 * c + 1]
        in_maps.append({
            "xT": xT, "wq": wq_c, "wkv": wkv_c, "wo": wo_c,
            "posT": posT, "wp1T": wp1T, "wp2T": wp2T,
            "whT2": whT2_c, "bp1c": bp1c, "bp2c": bp2c,
            "gate2a": g2a, "gate2b": g2b, "tabc": tabc, "tabs": tabs,
        })
    return in_maps


def get_program(reps=1):
    key = f"nc{reps}"
    if key not in _CACHE:
        _CACHE[key] = _build_program(reps)
    return _CACHE[key]


def kernel(**inputs) -> np.ndarray:
    nc = get_program()
    in_maps = _host_inputs(inputs)
    res = bass_utils.run_bass_kernel_spmd(nc, in_maps, list(range(NC)))
    out = np.zeros((T, DIM), np.float32)
    for c in range(NC):
        out += res.results[c]["outp"].astype(np.float32)
    out += np.asarray(inputs["bo"], np.float32)
    return out.reshape(1, T, DIM)

